# revision 17
# baseline (speedup 1.0000x reference)
"""Trainium2 Bass kernel for BERT self-attention.

Problem: B=16, S=512, H=1024, 16 heads x 64. Data-parallel over batch:
each of the 8 cores owns 2 batches and runs the full attention for them.

Per-core layout (T = 2*512 = 1024 local tokens), all-bf16 datapath:
  - xT  [H=1024, T=1024] bf16 : hidden states transposed (host-side)
  - wqT/wkT/wvT [H, O] bf16   : weights transposed (host-side)
  - QT, KT computed as [O, T] (transposed) in bf16: bias per-partition.
  - Q/K proj inner loop shares each stationary weight tile across the
    two moving t-halves (and V shares the stationary x tile across the
    two o-halves) so back-to-back LDWEIGHTS can be elided/overlapped.
  - V stored interleaved as [128, 16*(64+1)] bf16 with a ones-column per
    head; the ones-column turns the softmax denominator into one extra
    row of the context matmul.
  - attention mask folded in as a row-scaling of V' by exp(mask/8).
  - bv folded into the final output add (softmax rows sum to 1).
  - scoresT [keys, queries] per (b, h) in bf16; exp on ScalarE; ctxT' =
    V'.T @ expT; PE-transpose back to [queries, 64+1]; DVE: reciprocal
    of denom col, multiply, add bv; DMA out in four column chunks.
"""

import os
import sys

import numpy as np

if "/opt/trn_rl_repo" not in sys.path:
    sys.path.insert(0, "/opt/trn_rl_repo")

import ml_dtypes

BF16 = ml_dtypes.bfloat16

NCORES = 8
B = 16
S = 512
H = 1024
NH = 16
HS = 64
B_LOC = B // NCORES          # 2 batches per core
T = B_LOC * S                # 1024 tokens per core
NK = H // 128                # 8 contraction chunks

_prog_cache = {}
last_results = None          # BassKernelResults from the most recent run


def _ensure_ntff_hook():
    """Install antenv.axon_hooks if the image lacks it (profiling only)."""
    try:
        import antenv.axon_hooks  # noqa: F401
        return
    except ImportError:
        pass
    try:
        import types
        import antenv
        from trn_agent_boot.trn_boot import _ntff_profile_via_ctypes

        mod = types.ModuleType("antenv.axon_hooks")
        state = {"hook": None}
        mod.set_axon_ntff_profile_hook = lambda h: state.__setitem__("hook", h)
        mod.get_axon_ntff_profile_hook = lambda: state["hook"]
        sys.modules["antenv.axon_hooks"] = mod
        antenv.axon_hooks = mod
        hook = _ntff_profile_via_ctypes("/opt/axon/libaxon_pjrt.so")
        if hook is not None:
            mod.set_axon_ntff_profile_hook(hook)
    except Exception as e:  # profiling is best-effort
        print(f"ntff hook install failed: {e}", file=sys.stderr)


def _build_program():
    from concourse import bacc, mybir, tile
    import concourse.bass as bass

    f32 = mybir.dt.float32
    bf = mybir.dt.bfloat16
    Exp = mybir.ActivationFunctionType.Exp
    Ident = mybir.ActivationFunctionType.Identity

    nc = bacc.Bacc("TRN2", target_bir_lowering=False, debug=False,
                   enable_asserts=False)

    xT_d = nc.dram_tensor("xT", [H, T], bf, kind="ExternalInput").ap()
    wqT_d = nc.dram_tensor("wqT", [H, H], bf, kind="ExternalInput").ap()
    wkT_d = nc.dram_tensor("wkT", [H, H], bf, kind="ExternalInput").ap()
    wvT_d = nc.dram_tensor("wvT", [H, H], bf, kind="ExternalInput").ap()
    bq_d = nc.dram_tensor("bq2", [128, NK], f32, kind="ExternalInput").ap()
    bk_d = nc.dram_tensor("bk2", [128, NK], f32, kind="ExternalInput").ap()
    bvb_d = nc.dram_tensor("bvb", [128, H], f32, kind="ExternalInput").ap()
    maskw_d = nc.dram_tensor("maskw", [128, NK], f32, kind="ExternalInput").ap()
    ident_d = nc.dram_tensor("ident", [128, 128], bf, kind="ExternalInput").ap()
    out_d = nc.dram_tensor("out", [T, H], f32, kind="ExternalOutput").ap()

    with tile.TileContext(nc) as tc:
        with (
            tc.tile_pool(name="const", bufs=1) as const_pool,
            tc.tile_pool(name="persist", bufs=1) as persist,
            tc.tile_pool(name="outp", bufs=1) as outp,
            tc.tile_pool(name="xw", bufs=1) as xw_pool,
        ):
            # constants (DMAs emitted after the first x/w tiles below)
            ident_bf = const_pool.tile([128, 128], bf, name="ident_bf")
            bq_sb = const_pool.tile([128, NK], f32, name="bq_sb")
            bk_sb = const_pool.tile([128, NK], f32, name="bk_sb")
            bvb_sb = const_pool.tile([128, H], f32, name="bvb_sb")
            maskw_sb = const_pool.tile([128, NK], f32, name="maskw_sb")

            # activations (transposed), kept resident
            xts = [xw_pool.tile([128, T], bf, name=f"xt{k}", tag=f"xt{k}")
                   for k in range(NK)]

            qt_sb = [persist.tile([128, T], bf, name=f"qt{i}", tag=f"qt{i}")
                     for i in range(NK)]
            kt_sb = [persist.tile([128, T], bf, name=f"kt{i}", tag=f"kt{i}")
                     for i in range(NK)]
            # V' tiles: [128, 16 heads * 65]; col 64 of each head = ones*w
            vp_sb = [persist.tile([128, NH * (HS + 1)], bf, name=f"vp{i}",
                                  tag=f"vp{i}")
                     for i in range(NK)]
            ot_sb = [outp.tile([128, H], f32, name=f"ot{i}", tag=f"ot{i}")
                     for i in range(NK)]

            with (
                tc.tile_pool(name="wtile", bufs=1) as w_pool,
                tc.tile_pool(name="pproj", bufs=8, space="PSUM") as pproj,
            ):
                # interleave x / wq / wk tile loads so the first Q-proj
                # matmul's deps (xt0, wq0) land first; constants queue after.
                wq_t, wk_t, wv_t = [], [], []
                for k in range(NK):
                    nc.sync.dma_start(xts[k][:], xT_d[k * 128:(k + 1) * 128, :])
                    wq = w_pool.tile([128, H], bf, name=f"wq{k}", tag=f"wq{k}")
                    nc.sync.dma_start(wq[:], wqT_d[k * 128:(k + 1) * 128, :])
                    wq_t.append(wq)
                    wk = w_pool.tile([128, H], bf, name=f"wk{k}", tag=f"wk{k}")
                    nc.sync.dma_start(wk[:], wkT_d[k * 128:(k + 1) * 128, :])
                    wk_t.append(wk)
                nc.sync.dma_start(bq_sb[:], bq_d[:])
                nc.sync.dma_start(bk_sb[:], bk_d[:])
                nc.sync.dma_start(maskw_sb[:], maskw_d[:])
                nc.sync.dma_start(bvb_sb[:], bvb_d[:])
                nc.sync.dma_start(ident_bf[:], ident_d[:])

                # ---- QT / KT projections: out [o, t], bias per-partition ----
                # Inner order shares each stationary weight slice across the
                # two moving t-halves.
                for ot in range(NK):
                    o0 = ot * 128
                    ps = {}
                    for (pi, wt) in ((0, wq_t), (1, wk_t)):
                        for th in range(2):
                            ps[pi, th] = pproj.tile([128, 512], f32,
                                                    name="ps", tag="ps")
                    for k in range(NK):
                        st = (k == 0)
                        sp = (k == NK - 1)
                        for (pi, wt) in ((0, wq_t), (1, wk_t)):
                            w_sl = wt[k][:, o0:o0 + 128]
                            for th in range(2):
                                nc.tensor.matmul(
                                    ps[pi, th][:],
                                    w_sl,
                                    xts[k][:, th * 512:(th + 1) * 512],
                                    start=st, stop=sp,
                                )
                    for (pi, dst, bias_sb) in ((0, qt_sb, bq_sb),
                                               (1, kt_sb, bk_sb)):
                        for th in range(2):
                            nc.scalar.activation(
                                dst[ot][:, th * 512:(th + 1) * 512],
                                ps[pi, th][:], Ident,
                                bias=bias_sb[:, ot:ot + 1])

                # ---- V projection: natural [t, o] into interleaved V' ----
                wv_t = []
                for k in range(NK):
                    wv = w_pool.tile([128, H], bf, name=f"wv{k}", tag=f"wv{k}")
                    nc.sync.dma_start(wv[:], wvT_d[k * 128:(k + 1) * 128, :])
                    wv_t.append(wv)
                for tt in range(NK):
                    vv = vp_sb[tt].rearrange("p (h e) -> p h e", e=HS + 1)
                    t0 = tt * 128
                    ps0 = pproj.tile([128, 512], f32, name="ps", tag="ps")
                    ps1 = pproj.tile([128, 512], f32, name="ps", tag="ps")
                    for k in range(NK):
                        st = (k == 0)
                        sp = (k == NK - 1)
                        x_sl = xts[k][:, t0:t0 + 128]
                        nc.tensor.matmul(ps0[:], x_sl, wv_t[k][:, 0:512],
                                         start=st, stop=sp)
                        nc.tensor.matmul(ps1[:], x_sl, wv_t[k][:, 512:1024],
                                         start=st, stop=sp)
                    for oh, psx in ((0, ps0), (1, ps1)):
                        # evacuate with mask scaling: V'[k,:] *= exp(m_k/8)
                        nc.vector.tensor_scalar(
                            vv[:, oh * 8:(oh + 1) * 8, 0:HS],
                            psx.rearrange("p (h d) -> p h d", d=HS),
                            maskw_sb[:, tt:tt + 1], None,
                            mybir.AluOpType.mult,
                        )
                    # ones column, scaled by mask weight (= the weight itself)
                    nc.vector.tensor_copy(
                        vv[:, :, HS:HS + 1],
                        maskw_sb[:, tt:tt + 1].broadcast_to([128, NH, 1]))

            # ---- attention ----
            with (
                tc.tile_pool(name="ex", bufs=4) as ex_pool,
                tc.tile_pool(name="cs", bufs=4) as cs_pool,
                tc.tile_pool(name="rc", bufs=8) as rc_pool,
                tc.tile_pool(name="psc", bufs=2, space="PSUM") as sc_pool,
                tc.tile_pool(name="pcx", bufs=2, space="PSUM") as cx_pool,
                tc.tile_pool(name="ptr", bufs=2, space="PSUM") as tr_pool,
            ):
                for b in range(B_LOC):
                    for hp in range(NH // 2):
                        pair = (2 * hp, 2 * hp + 1)
                        qs = {}
                        exs = {}
                        for h in pair:
                            hb = (h % 2) * HS
                            qs[h] = qt_sb[hp][hb:hb + HS,
                                              b * 512:(b + 1) * 512]
                            exs[h] = ex_pool.tile([128, 2048], bf, name="ex",
                                                  tag="ex")
                        # scores for the head pair, interleaved so the two
                        # K=64 matmuls land in disjoint PE row groups and
                        # run concurrently.
                        for half in range(2):
                            scs = {h: sc_pool.tile([128, 1024], f32,
                                                   name="sc", tag="sc")
                                   for h in pair}
                            for j in range(2):
                                kt = half * 2 + j
                                c0 = b * 512 + kt * 128
                                for h in pair:
                                    hb = (h % 2) * HS
                                    nc.tensor.matmul(
                                        scs[h][:, j * 512:(j + 1) * 512],
                                        kt_sb[hp][hb:hb + HS, c0:c0 + 128],
                                        qs[h],
                                        start=True, stop=True,
                                    )
                            for h in pair:
                                nc.scalar.activation(
                                    exs[h][:, half * 1024:(half + 1) * 1024],
                                    scs[h][:], Exp, scale=0.125)
                        for h in pair:
                            ex = exs[h]
                            # ctxT' = V'.T @ expT  -> [65, 512] (row 64 = denom)
                            cx = cx_pool.tile([HS + 1, 512], f32, name="cx",
                                              tag="cx")
                            for kt in range(4):
                                vv = vp_sb[b * 4 + kt].rearrange(
                                    "p (h e) -> p h e", e=HS + 1)
                                nc.tensor.matmul(
                                    cx[:],
                                    vv[:, h, :],
                                    ex[:, kt * 512:(kt + 1) * 512],
                                    start=(kt == 0), stop=(kt == 3),
                                )
                            cs = cs_pool.tile([HS + 1, 512], bf, name="cs",
                                              tag="cs")
                            nc.vector.tensor_copy(cs[:], cx[:])
                            tr = tr_pool.tile([128, 4 * (HS + 2)], bf,
                                              name="tr", tag="tr")
                            trv = tr.rearrange("p (q e) -> p q e", e=HS + 2)
                            for qt in range(4):
                                nc.tensor.transpose(
                                    trv[:, qt, 0:HS + 1],
                                    cs[:, qt * 128:(qt + 1) * 128],
                                    ident_bf[0:HS + 1, 0:HS + 1])
                            rc = rc_pool.tile([128, 4, 1], f32, name="rc",
                                              tag="rc")
                            nc.vector.reciprocal(rc[:], trv[:, :, HS:HS + 1])
                            for qt in range(4):
                                osl = ot_sb[b * 4 + qt][:, h * HS:(h + 1) * HS]
                                # out = (ctx * 1/denom) + bv  in one DVE op
                                nc.vector.scalar_tensor_tensor(
                                    osl, trv[:, qt, 0:HS], rc[:, qt, :],
                                    bvb_sb[:, h * HS:(h + 1) * HS],
                                    mybir.AluOpType.mult, mybir.AluOpType.add)
                    for qt in range(4):
                        r0 = (b * 4 + qt) * 128
                        # eight column chunks (one per head pair): each
                        # drains as soon as its pair finishes, so only the
                        # last pair's 256KB remains after compute.
                        for cq in range(8):
                            c0 = cq * 128
                            nc.sync.dma_start(
                                out_d[r0:r0 + 128, c0:c0 + 128],
                                ot_sb[b * 4 + qt][:, c0:c0 + 128])

    nc.compile()
    return nc


def _get_program():
    if "nc" not in _prog_cache:
        _prog_cache["nc"] = _build_program()
    return _prog_cache["nc"]


def kernel(hidden_states, attention_mask, Wq, bq, Wk, bk, Wv, bv):
    global last_results
    from concourse import bass_utils

    hidden_states = np.ascontiguousarray(np.asarray(hidden_states,
                                                    dtype=np.float32))
    attention_mask = np.asarray(attention_mask, dtype=np.float32)
    Wq = np.asarray(Wq, dtype=np.float32)
    Wk = np.asarray(Wk, dtype=np.float32)
    Wv = np.asarray(Wv, dtype=np.float32)
    bq = np.asarray(bq, dtype=np.float32)
    bk = np.asarray(bk, dtype=np.float32)
    bv = np.asarray(bv, dtype=np.float32)

    nc = _get_program()

    wqT = np.ascontiguousarray(Wq.T).astype(BF16)
    wkT = np.ascontiguousarray(Wk.T).astype(BF16)
    wvT = np.ascontiguousarray(Wv.T).astype(BF16)
    bq2 = np.ascontiguousarray(bq.reshape(NK, 128).T)
    bk2 = np.ascontiguousarray(bk.reshape(NK, 128).T)
    bvb = np.ascontiguousarray(np.tile(bv[None, :], (128, 1)))
    ident = np.eye(128, dtype=np.float32).astype(BF16)

    mask = attention_mask.reshape(B, S)

    in_maps = []
    for c in range(NCORES):
        xT = np.ascontiguousarray(
            hidden_states[c * B_LOC:(c + 1) * B_LOC].reshape(T, H).T
        ).astype(BF16)
        # maskw[p, b*4+kt] = exp(mask[b, kt*128+p] / 8)
        mw = np.exp(mask[c * B_LOC:(c + 1) * B_LOC].reshape(B_LOC, 4, 128)
                    / 8.0).transpose(2, 0, 1).reshape(128, NK)
        in_maps.append({
            "xT": xT,
            "wqT": wqT, "wkT": wkT, "wvT": wvT,
            "bq2": bq2, "bk2": bk2,
            "bvb": bvb,
            "maskw": np.ascontiguousarray(mw.astype(np.float32)),
            "ident": ident,
        })

    trace = bool(os.environ.get("BASS_TRACE"))
    if trace:
        _ensure_ntff_hook()
    res = bass_utils.run_bass_kernel_spmd(
        nc, in_maps, core_ids=list(range(NCORES)), trace=trace,
    )
    last_results = res

    out = np.empty((B, S, H), dtype=np.float32)
    for c in range(NCORES):
        oc = res.results[c]["out"]
        out[c * B_LOC:(c + 1) * B_LOC] = oc.reshape(B_LOC, S, H)
    return out


# revision 18
# speedup vs baseline: 1.0148x; 1.0148x over previous
"""Trainium2 Bass kernel for BERT self-attention.

Problem: B=16, S=512, H=1024, 16 heads x 64. Data-parallel over batch:
each of the 8 cores owns 2 batches and runs the full attention for them.

Per-core layout (T = 2*512 = 1024 local tokens), all-bf16 datapath:
  - xT  [H=1024, T=1024] bf16 : hidden states transposed (host-side)
  - wqT/wkT/wvT [H, O] bf16   : weights transposed (host-side)
  - QT, KT computed as [O, T] (transposed) in bf16: bias per-partition.
  - Q/K proj inner loop shares each stationary weight tile across the
    two moving t-halves (and V shares the stationary x tile across the
    two o-halves) so back-to-back LDWEIGHTS can be elided/overlapped.
  - V stored interleaved as [128, 16*(64+1)] bf16 with a ones-column per
    head; the ones-column turns the softmax denominator into one extra
    row of the context matmul.
  - attention mask folded in as a row-scaling of V' by exp(mask/8).
  - bv folded into the final output add (softmax rows sum to 1).
  - scoresT [keys, queries] per (b, h) in bf16; exp on ScalarE; ctxT' =
    V'.T @ expT; PE-transpose back to [queries, 64+1]; DVE: reciprocal
    of denom col, multiply, add bv; DMA out in four column chunks.
"""

import os
import sys

import numpy as np

if "/opt/trn_rl_repo" not in sys.path:
    sys.path.insert(0, "/opt/trn_rl_repo")

import ml_dtypes

BF16 = ml_dtypes.bfloat16

NCORES = 8
B = 16
S = 512
H = 1024
NH = 16
HS = 64
B_LOC = B // NCORES          # 2 batches per core
T = B_LOC * S                # 1024 tokens per core
NK = H // 128                # 8 contraction chunks

_prog_cache = {}
last_results = None          # BassKernelResults from the most recent run


def _ensure_ntff_hook():
    """Install antenv.axon_hooks if the image lacks it (profiling only)."""
    try:
        import antenv.axon_hooks  # noqa: F401
        return
    except ImportError:
        pass
    try:
        import types
        import antenv
        from trn_agent_boot.trn_boot import _ntff_profile_via_ctypes

        mod = types.ModuleType("antenv.axon_hooks")
        state = {"hook": None}
        mod.set_axon_ntff_profile_hook = lambda h: state.__setitem__("hook", h)
        mod.get_axon_ntff_profile_hook = lambda: state["hook"]
        sys.modules["antenv.axon_hooks"] = mod
        antenv.axon_hooks = mod
        hook = _ntff_profile_via_ctypes("/opt/axon/libaxon_pjrt.so")
        if hook is not None:
            mod.set_axon_ntff_profile_hook(hook)
    except Exception as e:  # profiling is best-effort
        print(f"ntff hook install failed: {e}", file=sys.stderr)


def _build_program():
    from concourse import bacc, mybir, tile
    import concourse.bass as bass

    f32 = mybir.dt.float32
    bf = mybir.dt.bfloat16
    Exp = mybir.ActivationFunctionType.Exp
    Ident = mybir.ActivationFunctionType.Identity

    nc = bacc.Bacc("TRN2", target_bir_lowering=False, debug=False,
                   enable_asserts=False)

    xT_d = nc.dram_tensor("xT", [H, T], bf, kind="ExternalInput").ap()
    wqT_d = nc.dram_tensor("wqT", [H, H], bf, kind="ExternalInput").ap()
    wkT_d = nc.dram_tensor("wkT", [H, H], bf, kind="ExternalInput").ap()
    wvT_d = nc.dram_tensor("wvT", [H, H], bf, kind="ExternalInput").ap()
    bq_d = nc.dram_tensor("bq2", [128, NK], f32, kind="ExternalInput").ap()
    bk_d = nc.dram_tensor("bk2", [128, NK], f32, kind="ExternalInput").ap()
    bvb_d = nc.dram_tensor("bvb", [128, H], f32, kind="ExternalInput").ap()
    maskw_d = nc.dram_tensor("maskw", [128, NK], f32, kind="ExternalInput").ap()
    ident_d = nc.dram_tensor("ident", [128, 128], bf, kind="ExternalInput").ap()
    out_d = nc.dram_tensor("out", [T, H], f32, kind="ExternalOutput").ap()

    with tile.TileContext(nc) as tc:
        with (
            tc.tile_pool(name="const", bufs=1) as const_pool,
            tc.tile_pool(name="persist", bufs=1) as persist,
            tc.tile_pool(name="outp", bufs=1) as outp,
            tc.tile_pool(name="xw", bufs=1) as xw_pool,
        ):
            # constants (DMAs emitted after the first x/w tiles below)
            ident_bf = const_pool.tile([128, 128], bf, name="ident_bf")
            bq_sb = const_pool.tile([128, NK], f32, name="bq_sb")
            bk_sb = const_pool.tile([128, NK], f32, name="bk_sb")
            bvb_sb = const_pool.tile([128, H], f32, name="bvb_sb")
            maskw_sb = const_pool.tile([128, NK], f32, name="maskw_sb")

            # activations (transposed), kept resident
            xts = [xw_pool.tile([128, T], bf, name=f"xt{k}", tag=f"xt{k}")
                   for k in range(NK)]

            qt_sb = [persist.tile([128, T], bf, name=f"qt{i}", tag=f"qt{i}")
                     for i in range(NK)]
            kt_sb = [persist.tile([128, T], bf, name=f"kt{i}", tag=f"kt{i}")
                     for i in range(NK)]
            # V' tiles: [128, 16 heads * 65]; col 64 of each head = ones*w
            vp_sb = [persist.tile([128, NH * (HS + 1)], bf, name=f"vp{i}",
                                  tag=f"vp{i}")
                     for i in range(NK)]
            ot_sb = [outp.tile([128, H], f32, name=f"ot{i}", tag=f"ot{i}")
                     for i in range(NK)]

            with (
                tc.tile_pool(name="wtile", bufs=1) as w_pool,
                tc.tile_pool(name="pproj", bufs=8, space="PSUM") as pproj,
            ):
                # interleave x / wq / wk tile loads so the first Q-proj
                # matmul's deps (xt0, wq0) land first; constants queue after.
                wq_t, wk_t, wv_t = [], [], []
                for k in range(NK):
                    nc.sync.dma_start(xts[k][:], xT_d[k * 128:(k + 1) * 128, :])
                    wq = w_pool.tile([128, H], bf, name=f"wq{k}", tag=f"wq{k}")
                    nc.sync.dma_start(wq[:], wqT_d[k * 128:(k + 1) * 128, :])
                    wq_t.append(wq)
                    wk = w_pool.tile([128, H], bf, name=f"wk{k}", tag=f"wk{k}")
                    nc.sync.dma_start(wk[:], wkT_d[k * 128:(k + 1) * 128, :])
                    wk_t.append(wk)
                nc.sync.dma_start(bq_sb[:], bq_d[:])
                nc.sync.dma_start(bk_sb[:], bk_d[:])
                nc.sync.dma_start(maskw_sb[:], maskw_d[:])
                nc.sync.dma_start(bvb_sb[:], bvb_d[:])
                nc.sync.dma_start(ident_bf[:], ident_d[:])

                # ---- QT / KT projections: out [o, t], bias per-partition ----
                # Inner order shares each stationary weight slice across the
                # two moving t-halves.
                for ot in range(NK):
                    o0 = ot * 128
                    ps = {}
                    for (pi, wt) in ((0, wq_t), (1, wk_t)):
                        for th in range(2):
                            ps[pi, th] = pproj.tile([128, 512], f32,
                                                    name="ps", tag="ps")
                    for k in range(NK):
                        st = (k == 0)
                        sp = (k == NK - 1)
                        for (pi, wt) in ((0, wq_t), (1, wk_t)):
                            w_sl = wt[k][:, o0:o0 + 128]
                            for th in range(2):
                                nc.tensor.matmul(
                                    ps[pi, th][:],
                                    w_sl,
                                    xts[k][:, th * 512:(th + 1) * 512],
                                    start=st, stop=sp,
                                )
                    for (pi, dst, bias_sb) in ((0, qt_sb, bq_sb),
                                               (1, kt_sb, bk_sb)):
                        for th in range(2):
                            nc.scalar.activation(
                                dst[ot][:, th * 512:(th + 1) * 512],
                                ps[pi, th][:], Ident,
                                bias=bias_sb[:, ot:ot + 1])

                # ---- V projection: natural [t, o] into interleaved V' ----
                wv_t = []
                for k in range(NK):
                    wv = w_pool.tile([128, H], bf, name=f"wv{k}", tag=f"wv{k}")
                    nc.sync.dma_start(wv[:], wvT_d[k * 128:(k + 1) * 128, :])
                    wv_t.append(wv)
                for tt in range(NK):
                    vv = vp_sb[tt].rearrange("p (h e) -> p h e", e=HS + 1)
                    t0 = tt * 128
                    ps0 = pproj.tile([128, 512], f32, name="ps", tag="ps")
                    ps1 = pproj.tile([128, 512], f32, name="ps", tag="ps")
                    for k in range(NK):
                        st = (k == 0)
                        sp = (k == NK - 1)
                        x_sl = xts[k][:, t0:t0 + 128]
                        nc.tensor.matmul(ps0[:], x_sl, wv_t[k][:, 0:512],
                                         start=st, stop=sp)
                        nc.tensor.matmul(ps1[:], x_sl, wv_t[k][:, 512:1024],
                                         start=st, stop=sp)
                    for oh, psx in ((0, ps0), (1, ps1)):
                        # evacuate with mask scaling: V'[k,:] *= exp(m_k/8)
                        nc.vector.tensor_scalar(
                            vv[:, oh * 8:(oh + 1) * 8, 0:HS],
                            psx.rearrange("p (h d) -> p h d", d=HS),
                            maskw_sb[:, tt:tt + 1], None,
                            mybir.AluOpType.mult,
                        )
                    # ones column, scaled by mask weight (= the weight itself)
                    nc.vector.tensor_copy(
                        vv[:, :, HS:HS + 1],
                        maskw_sb[:, tt:tt + 1].broadcast_to([128, NH, 1]))

            # ---- attention ----
            with (
                tc.tile_pool(name="ex", bufs=4) as ex_pool,
                tc.tile_pool(name="cs", bufs=4) as cs_pool,
                tc.tile_pool(name="rc", bufs=8) as rc_pool,
                tc.tile_pool(name="psc", bufs=2, space="PSUM") as sc_pool,
                tc.tile_pool(name="pcx", bufs=2, space="PSUM") as cx_pool,
                tc.tile_pool(name="ptr", bufs=2, space="PSUM") as tr_pool,
            ):
                for b in range(B_LOC):
                    for hp in range(NH // 2):
                        pair = (2 * hp, 2 * hp + 1)
                        qs = {}
                        exs = {}
                        for h in pair:
                            hb = (h % 2) * HS
                            qs[h] = qt_sb[hp][hb:hb + HS,
                                              b * 512:(b + 1) * 512]
                            exs[h] = ex_pool.tile([128, 2048], bf, name="ex",
                                                  tag="ex")
                        # scores for the head pair, interleaved so the two
                        # K=64 matmuls land in disjoint PE row groups and
                        # run concurrently.
                        for half in range(2):
                            scs = {h: sc_pool.tile([128, 1024], f32,
                                                   name="sc", tag="sc")
                                   for h in pair}
                            for j in range(2):
                                kt = half * 2 + j
                                c0 = b * 512 + kt * 128
                                for h in pair:
                                    hb = (h % 2) * HS
                                    nc.tensor.matmul(
                                        scs[h][:, j * 512:(j + 1) * 512],
                                        kt_sb[hp][hb:hb + HS, c0:c0 + 128],
                                        qs[h],
                                        start=True, stop=True,
                                    )
                            for h in pair:
                                nc.scalar.activation(
                                    exs[h][:, half * 1024:(half + 1) * 1024],
                                    scs[h][:], Exp, scale=0.125)
                        for h in pair:
                            ex = exs[h]
                            # ctxT' = V'.T @ expT  -> [65, 512] (row 64 = denom)
                            cx = cx_pool.tile([HS + 1, 512], f32, name="cx",
                                              tag="cx")
                            for kt in range(4):
                                vv = vp_sb[b * 4 + kt].rearrange(
                                    "p (h e) -> p h e", e=HS + 1)
                                nc.tensor.matmul(
                                    cx[:],
                                    vv[:, h, :],
                                    ex[:, kt * 512:(kt + 1) * 512],
                                    start=(kt == 0), stop=(kt == 3),
                                )
                            cs = cs_pool.tile([HS + 1, 512], bf, name="cs",
                                              tag="cs")
                            nc.vector.tensor_copy(cs[:], cx[:])
                            tr = tr_pool.tile([128, 4 * (HS + 2)], bf,
                                              name="tr", tag="tr")
                            trv = tr.rearrange("p (q e) -> p q e", e=HS + 2)
                            for qt in range(4):
                                nc.tensor.transpose(
                                    trv[:, qt, 0:HS + 1],
                                    cs[:, qt * 128:(qt + 1) * 128],
                                    ident_bf[0:HS + 1, 0:HS + 1])
                            rc = rc_pool.tile([128, 4, 1], f32, name="rc",
                                              tag="rc")
                            nc.vector.reciprocal(rc[:], trv[:, :, HS:HS + 1])
                            for qt in range(4):
                                osl = ot_sb[b * 4 + qt][:, h * HS:(h + 1) * HS]
                                # out = (ctx * 1/denom) + bv  in one DVE op
                                nc.vector.scalar_tensor_tensor(
                                    osl, trv[:, qt, 0:HS], rc[:, qt, :],
                                    bvb_sb[:, h * HS:(h + 1) * HS],
                                    mybir.AluOpType.mult, mybir.AluOpType.add)
                    for qt in range(4):
                        r0 = (b * 4 + qt) * 128
                        # four column chunks: earlier head groups' chunks
                        # drain while later heads still compute.
                        for cq in range(4):
                            c0 = cq * 256
                            nc.sync.dma_start(
                                out_d[r0:r0 + 128, c0:c0 + 256],
                                ot_sb[b * 4 + qt][:, c0:c0 + 256])

    nc.compile()
    return nc


def _get_program():
    if "nc" not in _prog_cache:
        _prog_cache["nc"] = _build_program()
    return _prog_cache["nc"]


def kernel(hidden_states, attention_mask, Wq, bq, Wk, bk, Wv, bv):
    global last_results
    from concourse import bass_utils

    hidden_states = np.ascontiguousarray(np.asarray(hidden_states,
                                                    dtype=np.float32))
    attention_mask = np.asarray(attention_mask, dtype=np.float32)
    Wq = np.asarray(Wq, dtype=np.float32)
    Wk = np.asarray(Wk, dtype=np.float32)
    Wv = np.asarray(Wv, dtype=np.float32)
    bq = np.asarray(bq, dtype=np.float32)
    bk = np.asarray(bk, dtype=np.float32)
    bv = np.asarray(bv, dtype=np.float32)

    nc = _get_program()

    wqT = np.ascontiguousarray(Wq.T).astype(BF16)
    wkT = np.ascontiguousarray(Wk.T).astype(BF16)
    wvT = np.ascontiguousarray(Wv.T).astype(BF16)
    bq2 = np.ascontiguousarray(bq.reshape(NK, 128).T)
    bk2 = np.ascontiguousarray(bk.reshape(NK, 128).T)
    bvb = np.ascontiguousarray(np.tile(bv[None, :], (128, 1)))
    ident = np.eye(128, dtype=np.float32).astype(BF16)

    mask = attention_mask.reshape(B, S)

    in_maps = []
    for c in range(NCORES):
        xT = np.ascontiguousarray(
            hidden_states[c * B_LOC:(c + 1) * B_LOC].reshape(T, H).T
        ).astype(BF16)
        # maskw[p, b*4+kt] = exp(mask[b, kt*128+p] / 8)
        mw = np.exp(mask[c * B_LOC:(c + 1) * B_LOC].reshape(B_LOC, 4, 128)
                    / 8.0).transpose(2, 0, 1).reshape(128, NK)
        in_maps.append({
            "xT": xT,
            "wqT": wqT, "wkT": wkT, "wvT": wvT,
            "bq2": bq2, "bk2": bk2,
            "bvb": bvb,
            "maskw": np.ascontiguousarray(mw.astype(np.float32)),
            "ident": ident,
        })

    trace = bool(os.environ.get("BASS_TRACE"))
    if trace:
        _ensure_ntff_hook()
    res = bass_utils.run_bass_kernel_spmd(
        nc, in_maps, core_ids=list(range(NCORES)), trace=trace,
    )
    last_results = res

    out = np.empty((B, S, H), dtype=np.float32)
    for c in range(NCORES):
        oc = res.results[c]["out"]
        out[c * B_LOC:(c + 1) * B_LOC] = oc.reshape(B_LOC, S, H)
    return out
